# revision 14
# baseline (speedup 1.0000x reference)
"""Trainium2 Bass kernel for nn_Decoder_36636071035490.

Reference computes, for workers i and task/edge (j,l):
    z = worker_feature @ W            # [2000, 1]
    p1 = sigmoid(z + b)
    p2 = (1 - p1) / 9
    P[i, j, l] = p1_i^tau_jl * p2_i^(1 - tau_jl)      # [2000, 5000, 10] f32

Identity used on device (exact in exact arithmetic):
    P[i, f] = exp(a_i * tau_f + c_i)
    a_i = (z_i + b) + ln 9            # since logit(sigmoid(x)) = x
    c_i = -ln(1 + exp(z_i + b)) - ln 9

Output is stored as bf16 (rel-err budget 2e-2 >> bf16 rounding); the host
upcasts to f32.  The kernel is store-bound: 25 MB of bf16 stores per core.
DMA layout rules learned from traces on this part:
 * The descriptor spreader assigns a transfer's per-partition
   descriptors to the largest divisor <= 16 of the partition count, in
   contiguous blocks starting at engine 0 (measured: 128 parts -> 16
   engines, 125 -> 5, 124 -> 4).  Partition-offset sources also blow
   HWDGE issue time from 0.6us to 1.4-14us per store.  So every DMA here
   is partition-0-aligned.
 * SDMA engine 15 moves ~18% fewer bytes/sec than engines 0-14 (the
   dynamic-queue control rings live on it) and would end the kernel ~11us
   after everyone else.  Worker tiles are therefore 120 ROWS: every
   [120, cols] transfer spreads over engines 0-14 x 8 descriptors and
   engine 15 carries nothing.  17 tiles (16 aligned + one overlapping
   tail that re-writes 40 rows with identical bytes); stores are
   one-per-tile [120, 6250] with 12.5 KB descriptors (ACT and PE columns
   share a staging tile).

Columns split between the two 1-elem/cycle producers:
 * ACT path (NA=3584): ScalarE exp(a*tau+c) per worker tile, bf16 out.
 * PE path (NP=2666): rank-12 Chebyshev-Lagrange factorization in
   d_i = z_i + b (range ~+-0.3), hi/lo split along K (K=24):
       P[i,f] = r_i * sum_m L_m[i] * exp((node_m + ln9) * tau_f + ln|w_m|)
   TensorE matmuls (512-col, bf16) -> PSUM groups (1536, 1130); VectorE
   drains PSUM->SBUF as a tensor_scalar multiply by r_i (PSUM has one DVE
   read port -> drains are 1x regardless).  Split keeps ScalarE (~60us)
   ~= VectorE (~64us) < DMA.

Ramp/tail discipline (the ~7us engine-init preamble is fixed cost):
 * cst rides inside the wk DMA ([124, 1088+CW] fat load, 4.7 KB
   descriptors) -- a standalone [124, CW] cst load has 300 B descriptors
   and takes ~3us to land, gating the whole scalar chain.
 * Single-pass per-worker scalars for all 17 tiles (a split fast path
   tempts the Tile scheduler into queue-jumping the batch's Exp ahead of
   the first ACT tiles).  tau for the first ACT chunk loads first.
 * V build goes right after the first ACT chunk so the PE pipeline
   (transposes -> matmuls -> drains) starts ~20us in.
 * Tiles 0-1 store their ACT columns separately (the PE pipeline is not
   up yet); tiles 2+ use the unified per-tile store.

ScalarE only ever evaluates Exp (c_i comes from a 5-term ln(1+t) poly on
VectorE, t = (e^d-1)/2) -> one ACT table load.  U-build constants (ones,
sgn) come from host-packed cst columns via stride-0 APs -> no GpSimd at
all (a GpSimd custom op costs a Q7 library load and its SBUF streaming
fights DVE for ports; measured as a 10x slowdown of concurrent DVE ops).

Sharding: by output columns (task*edge flattened, 50000 -> 8 x 6250);
every core computes the per-worker scalars for all 2000 workers
(replicated) and produces the full-height [2000, 6250] slab.
"""

import numpy as np

WORKERS = 2000
TASKS = 5000
ET = 10
AB = 64
NCORES = 8
F = TASKS * ET  # 50000 output cols
FS = F // NCORES  # 6250 cols per core
LN9 = float(np.log(9.0))
LN18 = float(np.log(18.0))

NA = 3584  # ACT-path cols per core
NP = FS - NA  # 2666 PE-path cols per core
G1, G2 = 1536, NP - 1536  # PSUM drain groups
RANK = 12
KTOT = 2 * RANK  # contraction rows: [U1 | U2] x [V1 | V1]
DLIM = 0.5

# Chebyshev nodes and barycentric-style weights (sign folded into U via a
# host-packed constant column, the magnitude ln|w| - ln18 into V's ACT bias)
_m = np.arange(RANK)
_NODES = (DLIM * np.cos((2 * _m + 1) / (2 * RANK) * np.pi)).astype(np.float64)
_WTS = np.array(
    [
        1.0 / np.prod([_NODES[m] - _NODES[j] for j in range(RANK) if j != m])
        for m in range(RANK)
    ]
)
_SGN = np.sign(_WTS)
_LNW = np.log(np.abs(_WTS)) - LN18

# 120-row worker tiles: 16 aligned + one overlapping tail tile (stored
# full-width; the 40 overlap rows are double-written with identical data)
PR = 120
NT = 17
_WSTARTS = [PR * t for t in range(NT - 1)] + [WORKERS - PR]
IDW = 256  # ident padded to 512B descriptors (exactly the line-rate minimum)

# packed constant layout, appended to the wk fat load: [PR, CW] f32
#   cols 0:AB          W broadcast down partitions
#   col  AB            b
#   col  AB+1          snod (rows 0:KTOT)  = node_m + ln9, twice
#   col  AB+2          lnw  (rows 0:KTOT)  = ln|w_m| - ln18, twice
#   col  AB+3          1.0 (ones column for pre/suf init)
#   cols AB+4:AB+4+R   sgn_j broadcast down partitions
CW = AB + 4 + RANK
WKW = NT * AB  # wk cols in the fat load
FATW = WKW + CW

_CACHE = {}


def _build_nc():
    import concourse.bass as bass
    import concourse.mybir as mybir
    from concourse import bacc
    from concourse.tile import TileContext
    from contextlib import ExitStack

    f32 = mybir.dt.float32
    bf16 = mybir.dt.bfloat16
    AF = mybir.ActivationFunctionType
    OP = mybir.AluOpType

    nc = bacc.Bacc("TRN2")
    # worker features pre-arranged on host to [PR, tile, AB], constants
    # appended -> one contiguous 4.7KB-descriptor DMA
    fat = nc.dram_tensor("fat", [PR, FATW], f32, kind="ExternalInput")
    # ACT-path tau cols, pre-replicated across PR partitions
    tfa = nc.dram_tensor("tfa", [PR, NA], f32, kind="ExternalInput")
    # PE-path tau cols, replicated across KTOT partitions
    tfp = nc.dram_tensor("tfp", [KTOT, NP], f32, kind="ExternalInput")
    ident = nc.dram_tensor("ident", [PR, IDW], bf16, kind="ExternalInput")
    out = nc.dram_tensor("out", [WORKERS, FS], bf16, kind="ExternalOutput")

    with TileContext(nc) as tc, ExitStack() as ctx:
        const = ctx.enter_context(tc.tile_pool(name="const", bufs=1))
        stage_a = ctx.enter_context(tc.tile_pool(name="stagea", bufs=2))
        stage_p = ctx.enter_context(tc.tile_pool(name="stagep", bufs=2))
        stage_u = ctx.enter_context(tc.tile_pool(name="stageu", bufs=4))
        psum_p = ctx.enter_context(tc.tile_pool(name="psump", bufs=1, space="PSUM"))

        # ---- input loads; first ACT chunk's tau right after the fat load
        CH0 = 1792
        fatt = const.tile([PR, FATW], f32, name="fatt")
        nc.sync.dma_start(out=fatt, in_=fat[:])
        taub = const.tile([PR, NA], f32, name="taub")
        nc.sync.dma_start(out=taub[:, 0:CH0], in_=tfa[:, 0:CH0])
        taup = const.tile([KTOT, NP], f32, name="taup")
        nc.sync.dma_start(out=taup, in_=tfp[:])
        nc.sync.dma_start(out=taub[:, CH0:NA], in_=tfa[:, CH0:NA])
        idc = const.tile([PR, IDW], bf16, name="idc")
        nc.sync.dma_start(out=idc, in_=ident[:])

        wka = fatt[:, 0:WKW].rearrange("p (t a) -> p t a", a=AB)
        Wb = fatt[:, WKW : WKW + AB]
        bcol = fatt[:, WKW + AB : WKW + AB + 1]
        snodc = fatt[0:KTOT, WKW + AB + 1 : WKW + AB + 2]
        lnwc = fatt[0:KTOT, WKW + AB + 2 : WKW + AB + 3]
        onec = fatt[:, WKW + AB + 3 : WKW + AB + 4]
        sgnc = fatt[:, WKW + AB + 4 : WKW + AB + 4 + RANK]

        # ---- per-worker scalars for all 17 tiles in one pass:
        # z -> a (scale), c (bias), d = z+b, r = 2/(1+e^d) (the 1/18 lives
        # in V's bias).  c = -ln(1+t) - ln18 with t = (e^d-1)/2 via a
        # degree-5 poly on DVE, so ScalarE never needs the Ln table.
        dall = const.tile([PR, NT], f32, name="dall")
        aall = const.tile([PR, NT], f32, name="aall")
        cball = const.tile([PR, NT], f32, name="cball")
        eCall = const.tile([PR, NT], f32, name="eCall")
        WbT = bass.AP(
            tensor=Wb.tensor,
            offset=Wb.offset,
            ap=[list(Wb.ap[0]), [0, NT], [1, AB]],
        )
        proda = const.tile([PR, NT, AB], f32, name="proda")
        nc.vector.tensor_mul(proda, wka, WbT)
        zb_ = const.tile([PR, NT], f32, name="zb")
        nc.vector.reduce_sum(
            out=zb_.rearrange("p (t o) -> p t o", o=1),
            in_=proda,
            axis=mybir.AxisListType.X,
        )
        nc.vector.tensor_scalar(
            out=aall, in0=zb_, scalar1=bcol, scalar2=LN9, op0=OP.add, op1=OP.add
        )
        nc.vector.tensor_scalar_add(out=dall, in0=zb_, scalar1=bcol)
        eb_ = const.tile([PR, NT], f32, name="eb")
        nc.scalar.activation(out=eb_, in_=zb_, func=AF.Exp, bias=bcol, scale=1.0)
        tt_ = const.tile([PR, NT], f32, name="tt")
        nc.vector.tensor_scalar(
            out=tt_, in0=eb_, scalar1=0.5, scalar2=-0.5, op0=OP.mult, op1=OP.add
        )
        ut_ = const.tile([PR, NT], f32, name="ut")
        nc.vector.tensor_scalar_add(out=ut_, in0=tt_, scalar1=1.0)
        nc.vector.reciprocal(eCall, ut_)
        # ln(1+t) = t - t^2/2 + t^3/3 - t^4/4 + t^5/5 via chained
        # f <- (f + a_k) * t
        hs = const.tile([PR, 5, NT], f32, name="hs")
        nc.vector.tensor_scalar_mul(out=hs[:, 0, :], in0=tt_, scalar1=0.2)
        for k, ak in enumerate((-0.25, 1.0 / 3.0, -0.5, 1.0)):
            nc.vector.scalar_tensor_tensor(
                out=hs[:, k + 1, :], in0=hs[:, k, :], scalar=ak, in1=tt_,
                op0=OP.add, op1=OP.mult,
            )
        nc.vector.tensor_scalar(
            out=cball, in0=hs[:, 4, :], scalar1=-1.0, scalar2=-LN18,
            op0=OP.mult, op1=OP.add,
        )
        acol = [aall[:, t : t + 1] for t in range(NT)]
        ccol = [cball[:, t : t + 1] for t in range(NT)]
        eCc = [eCall[:, t : t + 1] for t in range(NT)]

        # ---- tile 0 ACT in 2 chunks (store stream starts ASAP), then the
        # V build (unblocks the whole PE pipeline), then tile 1.
        stgA0 = stage_a.tile([PR, NA], bf16, name="sA0", tag="sA")
        nc.scalar.activation(
            out=stgA0[:, 0:CH0], in_=taub[:, 0:CH0], func=AF.Exp,
            bias=ccol[0], scale=acol[0],
        )
        nc.sync.dma_start(out=out[0:PR, 0:CH0], in_=stgA0[:, 0:CH0])
        vt = const.tile([KTOT, NP], bf16, name="vt")
        nc.scalar.activation(out=vt, in_=taup, func=AF.Exp, bias=lnwc, scale=snodc)
        nc.scalar.activation(
            out=stgA0[:, CH0:NA], in_=taub[:, CH0:NA], func=AF.Exp,
            bias=ccol[0], scale=acol[0],
        )
        nc.sync.dma_start(out=out[0:PR, CH0:NA], in_=stgA0[:, CH0:NA])

        # ---- U build (VectorE): U = sgn * prefix*suffix products of
        # (d - node_j).  pre[0]/suf[RANK-1] come from the ones column and
        # the sign from sgn cols, both via stride-0 APs.
        dstk = const.tile([PR, RANK, NT], f32, name="dstk")
        pre = const.tile([PR, RANK, NT], f32, name="pre")
        suf = const.tile([PR, RANK, NT], f32, name="suf")
        ls_ = const.tile([PR, RANK, NT], f32, name="ls")
        ust = const.tile([PR, RANK, NT], f32, name="ust")
        upk = const.tile([PR, KTOT, NT], bf16, name="upk")
        uhi = const.tile([PR, RANK, NT], f32, name="uhi")
        utall = const.tile([KTOT, NT, PR], bf16, name="utall")
        one_nt = bass.AP(
            tensor=onec.tensor, offset=onec.offset,
            ap=[list(onec.ap[0]), [0, NT]],
        )
        sgn_bc = bass.AP(
            tensor=sgnc.tensor, offset=sgnc.offset,
            ap=[list(sgnc.ap[0]), [1, RANK], [0, NT]],
        )
        for j in range(RANK):
            nc.vector.tensor_scalar_add(
                out=dstk[:, j, :], in0=dall, scalar1=float(-_NODES[j])
            )
        nc.vector.tensor_copy(pre[:, 0, :], one_nt)
        nc.vector.tensor_copy(suf[:, RANK - 1, :], one_nt)
        for j in range(1, RANK):
            nc.vector.tensor_mul(pre[:, j, :], pre[:, j - 1, :], dstk[:, j - 1, :])
        for j in range(RANK - 2, -1, -1):
            nc.vector.tensor_mul(suf[:, j, :], suf[:, j + 1, :], dstk[:, j + 1, :])
        nc.vector.tensor_mul(ls_, pre, suf)
        nc.vector.tensor_mul(ust, ls_, sgn_bc)
        # hi/lo split packed [U1 | U2] along the free dim
        nc.vector.tensor_copy(upk[:, 0:RANK, :], ust)
        nc.vector.tensor_copy(uhi, upk[:, 0:RANK, :])
        nc.vector.tensor_sub(upk[:, RANK : 2 * RANK, :], ust, uhi)
        # transpose to [KTOT, PR] per tile via TensorE (3 batches, each
        # <=6*124 cols = 1 PSUM bank)
        # each tile's transpose lands at a 128-col (256 B) stride inside
        # psT: PSUM accesses must be 4-byte aligned and 125 bf16 = 250 B
        for b0 in range(0, NT, 6):
            b1 = min(NT, b0 + 6)
            nb = b1 - b0
            psT = psum_p.tile([KTOT, 6, 128], bf16, name=f"psT{b0}", tag="psT",
                              bufs=2)
            for k in range(nb):
                nc.tensor.transpose(
                    out=psT[:, k, 0:PR], in_=upk[:, :, b0 + k],
                    identity=idc[:, 0:PR],
                )
            nc.vector.tensor_copy(
                utall[:, b0:b1, :], psT[:, 0:nb, 0:PR]
            )

        # tile 1 ACT (program-ordered here; runs on ScalarE during U build)
        stgA1 = stage_a.tile([PR, NA], bf16, name="sA1", tag="sA")
        nc.scalar.activation(
            out=stgA1, in_=taub, func=AF.Exp, bias=ccol[1], scale=acol[1]
        )
        nc.sync.dma_start(out=out[_WSTARTS[1] : _WSTARTS[1] + PR, 0:NA],
                          in_=stgA1)

        # ---- main loop.  Tiles 0-1: PE cols go to their own staging
        # (their ACT cols already stored); tiles 2+: ACT (ScalarE) and PE
        # drains (VectorE) share one staging tile, stored in a single
        # [128, 6250] DMA (12.5 KB descriptors).
        def pe_cols(t, dst):
            off = 0
            for g, gs in enumerate((G1, G2)):
                pmm = psum_p.tile([PR, G1], f32, name=f"pmm{t}_{g}", tag="pmm",
                                  bufs=2)
                nmm = (gs + 511) // 512
                for j in range(nmm):
                    n0 = j * 512
                    n1 = min(gs, n0 + 512)
                    nc.tensor.matmul(
                        out=pmm[:, n0:n1],
                        lhsT=utall[:, t, :],
                        rhs=vt[:, off + n0 : off + n1],
                        start=True,
                        stop=True,
                    )
                nc.vector.tensor_scalar_mul(
                    out=dst[:, off : off + gs], in0=pmm[:, 0:gs], scalar1=eCc[t]
                )
                off += gs

        for t in (0, 1):
            stgP = stage_p.tile([PR, NP], bf16, name=f"sP{t}", tag="sP")
            pe_cols(t, stgP)
            w0 = _WSTARTS[t]
            nc.sync.dma_start(out=out[w0 : w0 + PR, NA:FS], in_=stgP)
        for t in range(2, NT):
            stgU = stage_u.tile([PR, FS], bf16, name=f"sU{t}", tag="sU")
            nc.scalar.activation(
                out=stgU[:, 0:NA], in_=taub, func=AF.Exp, bias=ccol[t],
                scale=acol[t],
            )
            pe_cols(t, stgU[:, NA:FS])
            w0 = _WSTARTS[t]
            nc.sync.dma_start(out=out[w0 : w0 + PR, :], in_=stgU)
    nc.compile()
    return nc


def _get_nc():
    if "nc" not in _CACHE:
        _CACHE["nc"] = _build_nc()
    return _CACHE["nc"]


def _make_in_maps(inputs_arr, W, b):
    import ml_dtypes

    wk0 = np.asarray(inputs_arr[:WORKERS, :AB], dtype=np.float32)
    W = np.asarray(W, dtype=np.float32).reshape(AB)
    b = np.asarray(b, dtype=np.float32).reshape(())
    fat = np.zeros((PR, FATW), np.float32)
    for t, ws in enumerate(_WSTARTS):
        fat[:, t * AB : (t + 1) * AB] = wk0[ws : ws + PR, :]
    nod32 = (_NODES + LN9).astype(np.float32)
    lnw32 = _LNW.astype(np.float32)
    fat[:, WKW : WKW + AB] = W[None, :]
    fat[:, WKW + AB] = b
    fat[0:KTOT, WKW + AB + 1] = np.concatenate([nod32, nod32])
    fat[0:KTOT, WKW + AB + 2] = np.concatenate([lnw32, lnw32])
    fat[:, WKW + AB + 3] = 1.0
    fat[:, WKW + AB + 4 : WKW + AB + 4 + RANK] = _SGN.astype(np.float32)[None, :]
    fat = np.ascontiguousarray(fat)
    tau_flat = np.ascontiguousarray(
        inputs_arr[WORKERS:, :ET], dtype=np.float32
    ).reshape(F)
    ident = np.zeros((PR, IDW), dtype=ml_dtypes.bfloat16)
    ident[:, 0:PR] = np.eye(PR, dtype=ml_dtypes.bfloat16)
    maps = []
    for c in range(NCORES):
        sl = tau_flat[c * FS : (c + 1) * FS]
        tfa = np.ascontiguousarray(np.broadcast_to(sl[0:NA], (PR, NA)))
        tfp = np.ascontiguousarray(np.broadcast_to(sl[NA:FS], (KTOT, NP)))
        maps.append(
            {
                "fat": fat,
                "tfa": tfa,
                "tfp": tfp,
                "ident": ident,
            }
        )
    return maps


def _run(inputs_arr, W, b, **kwargs):
    from concourse import bass_utils

    nc = _get_nc()
    in_maps = _make_in_maps(inputs_arr, W, b)
    return bass_utils.run_bass_kernel_spmd(
        nc, in_maps, core_ids=list(range(NCORES)), **kwargs
    )


def kernel(inputs, W, b):
    inputs_arr = np.asarray(inputs, dtype=np.float32)
    last_err = None
    for _ in range(3):  # retry transient device failures
        try:
            res = _run(inputs_arr, np.asarray(W), np.asarray(b))
            break
        except Exception as e:  # noqa: BLE001
            last_err = e
    else:
        raise last_err
    out = np.concatenate(
        [np.asarray(r["out"]).astype(np.float32) for r in res.results], axis=1
    )
    return out.reshape(WORKERS, TASKS, ET)
